# revision 11
# baseline (speedup 1.0000x reference)
"""Trainium2 Bass kernel for windowed Bahdanau (additive) attention.

Per sample b (B=256, S=512, H=1024, W=64):
    Mt  = inputs[b, len_b-W:len_b, :]                      # [W, H] window
    par = inputs[b, len_b-parent_b-1, :]                   # a row INSIDE the window
    energy = tanh([hc_b | Mt] @ W_attn.T + b_attn)         # [W, H]
    scores = energy @ v ; attn = softmax(scores)           # [W]
    context = attn @ Mt                                    # [H]

Sharding: pure data parallelism over batch (32 samples per core, 8 cores).
Host does the (index-based) window gather + layout transposes; all FLOPs run
on device.  Device layout is "transposed": energy kept as [ho, (b,w)] so the
dominant matmul (Mt @ W2.T, contraction over hi=1024) streams 512-wide moving
operands with the weights stationary.
"""

import os
import sys

sys.path.insert(0, "/opt/trn_rl_repo")

# The axon NTFF profile hook (antenv.axon_hooks) is absent in this container;
# run_bass_kernel_spmd's trace path would crash on import if BASS_TRACE leaked
# in from the environment.
os.environ["BASS_NEVER_TRACE"] = "1"

import numpy as np

import concourse.bass as bass
import concourse.tile as tile
from concourse import bacc, mybir
from concourse.bass import IndirectOffsetOnAxis
from concourse.bass_utils import run_bass_kernel_spmd
from concourse.masks import make_identity

B, S, H, W = 256, 512, 1024, 64
NCORES = 8
BC = B // NCORES          # 32 samples per core
BW = BC * W               # 2048 flattened (sample, window) columns per core
KC = H // 128             # 8 contraction (hi) chunks
HOC = H // 128            # 8 output (ho) chunks
NQ = BW // 512            # 4 column quarters

F32 = mybir.dt.float32
F32R = mybir.dt.float32r
I32 = mybir.dt.int32

# float32r streams the PE at 1 cycle/row (vs 4 for full fp32).  Walrus
# requires fp32r matmul operands to be *typed* float32r end-to-end (DMA-loaded
# f32r tensors and ACT outputs with f32r dtype both pass the BIR verifier).
# Flipped to False this kernel runs everything in exact fp32.
USE_F32R = True
MMDT = F32R if USE_F32R else F32


def _build_program():
    nc = bacc.Bacc(
        "TRN2",
        target_bir_lowering=False,
        debug=False,
        enable_asserts=False,
        num_devices=NCORES,
    )

    # ---- DRAM I/O ----------------------------------------------------------
    # per-core window data, transposed+chunked on host: wt[k, c, p, j] =
    # window[(b,w) = c*512+j, hi = k*128+p]
    wt_d = nc.dram_tensor("wt", [KC, NQ, 128, 512], MMDT, kind="ExternalInput").ap()
    # natural-layout windows, only the 32 `par` rows are ever read
    wnat_d = nc.dram_tensor("wnat", [BW, H], F32, kind="ExternalInput").ap()
    # hc transposed+chunked: hct[p, k, b] = hc[b, k*128+p]
    hct_d = nc.dram_tensor("hct", [128, KC, BC], MMDT, kind="ExternalInput").ap()
    pidx_d = nc.dram_tensor("pidx", [BC, 1], I32, kind="ExternalInput").ap()
    w1_d = nc.dram_tensor("w1c", [KC, 128, H], MMDT, kind="ExternalInput").ap()
    w2_d = nc.dram_tensor("w2c", [KC, 128, H], MMDT, kind="ExternalInput").ap()
    bias_d = nc.dram_tensor("biasc", [128, HOC], F32, kind="ExternalInput").ap()
    v_d = nc.dram_tensor("vc", [128, HOC], MMDT, kind="ExternalInput").ap()

    attn_d = nc.dram_tensor("attn_o", [BC, W], F32, kind="ExternalOutput").ap()
    ctx_d = nc.dram_tensor("ctx_o", [BC, H], F32, kind="ExternalOutput").ap()
    par_d = nc.dram_tensor("par_o", [BC, H], F32, kind="ExternalOutput").ap()

    with tile.TileContext(nc) as tc:
        _emit(tc, locals())
    nc.compile()
    return nc


def _emit(tc, io):
    nc = tc.nc
    from contextlib import ExitStack

    with ExitStack() as ctx:
        consts = ctx.enter_context(tc.tile_pool(name="consts", bufs=1))
        mtp = ctx.enter_context(tc.tile_pool(name="mtp", bufs=1))
        w2p = ctx.enter_context(tc.tile_pool(name="w2p", bufs=1))
        w1p = ctx.enter_context(tc.tile_pool(name="w1p", bufs=2))
        eqp = ctx.enter_context(tc.tile_pool(name="eqp", bufs=6))
        ctp = ctx.enter_context(tc.tile_pool(name="ctp", bufs=3))
        psq = ctx.enter_context(tc.tile_pool(name="psq", bufs=4, space="PSUM"))
        pss = ctx.enter_context(tc.tile_pool(name="pss", bufs=1, space="PSUM"))

        # ---- small constants ------------------------------------------------
        bias_sb = consts.tile([128, HOC], F32, name="bias_sb")
        nc.sync.dma_start(out=bias_sb, in_=io["bias_d"])
        v_sb = consts.tile([128, HOC], MMDT, name="v_sb")
        nc.sync.dma_start(out=v_sb, in_=io["v_d"])
        hct_sb = consts.tile([128, KC, BC], MMDT, name="hct_sb")
        nc.sync.dma_start(out=hct_sb, in_=io["hct_d"])
        ones_sb = consts.tile([1, 128], F32, name="ones_sb")
        nc.vector.memset(ones_sb, 1.0)
        ident_sb = consts.tile([128, 128], F32, name="ident_sb")
        make_identity(nc, ident_sb)
        pidx_sb = consts.tile([BC, 1], I32, name="pidx_sb")
        nc.sync.dma_start(out=pidx_sb, in_=io["pidx_d"])

        # ---- par output: gather 32 rows out of the natural windows ---------
        par_sb = consts.tile([BC, H], F32, name="par_sb")
        nc.gpsimd.indirect_dma_start(
            out=par_sb,
            out_offset=None,
            in_=io["wnat_d"],
            in_offset=IndirectOffsetOnAxis(ap=pidx_sb[:, :1], axis=0),
        )
        nc.sync.dma_start(out=io["par_d"], in_=par_sb)

        # ---- big loads (w1 first: hcW1 gates every tanh) --------------------
        # hcW1 = hc @ W1.T  (natural [b, ho] layout), w1 streamed through a
        # 2-deep pool so its DMAs lead the queue.
        hc_ps = []
        for h2 in range(2):
            t = psq.tile([BC, 512], F32, tag="eq", name=f"hcps{h2}")
            hc_ps.append(t)
        for k in range(KC):
            w1t = w1p.tile([128, H], MMDT, tag="w1", name=f"w1sb{k}")
            nc.sync.dma_start(out=w1t, in_=io["w1_d"][k])
            for h2 in range(2):
                nc.tensor.matmul(
                    out=hc_ps[h2],
                    lhsT=hct_sb[:, k, :],
                    rhs=w1t[:, h2 * 512:(h2 + 1) * 512],
                    start=(k == 0),
                    stop=(k == KC - 1),
                )
        hcw1_sb = consts.tile([BC, H], F32, name="hcw1_sb")
        for h2 in range(2):
            nc.scalar.copy(out=hcw1_sb[:, h2 * 512:(h2 + 1) * 512], in_=hc_ps[h2])
        # transpose to [ho, b] so the per-sample term broadcasts along free
        hcw1t_sb = consts.tile([128, HOC, BC], F32, name="hcw1t_sb")
        for h in range(HOC):
            tph = psq.tile([128, BC], F32, tag="eq", name=f"tph{h}")
            nc.tensor.transpose(
                out=tph, in_=hcw1_sb[:, h * 128:(h + 1) * 128],
                identity=ident_sb[:BC, :BC],
            )
            nc.scalar.copy(out=hcw1t_sb[:, h, :], in_=tph)

        w2t = []
        for k in range(KC):
            t = w2p.tile([128, H], MMDT, tag=f"w2_{k}", name=f"w2sb{k}")
            w2t.append(t)
        for half in range(2):
            sl = slice(half * 512, (half + 1) * 512)
            for k in range(KC):
                nc.sync.dma_start(out=w2t[k][:, sl], in_=io["w2_d"][k, :, sl])

        mt = {}
        for c in range(NQ):
            for k in range(KC):
                t = mtp.tile([128, 512], MMDT, tag=f"mt{k}_{c}", name=f"mt{k}_{c}")
                nc.sync.dma_start(out=t, in_=io["wt_d"][k, c])
                mt[k, c] = t

        # ---- energy (transposed layout) + scores ----------------------------
        # energyT[ho, (b,w)] accumulated per (ho-chunk h, column quarter c):
        #   sum_k W2T[k,h-slice].T @ MtT[k, c-quarter]   (K=128 each)
        # then the per-sample hc term is broadcast-added along the 64 window
        # columns on the DVE (0-stride AP), tanh+bias evacuates PSUM->SBUF on
        # ACT (rounding to f32r), and the v-dot accumulates scores on the PE.
        scores_ps = pss.tile([1, BW], F32, tag="scores", name="scores_ps")
        for h in range(HOC):
            hsl = slice(h * 128, (h + 1) * 128)
            for c in range(NQ):
                csl = slice(c * 512, (c + 1) * 512)
                eps = psq.tile([128, 512], F32, tag="eq", name=f"eps{h}_{c}")
                for k in range(KC):
                    nc.tensor.matmul(
                        out=eps,
                        lhsT=w2t[k][:, hsl],
                        rhs=mt[k, c],
                        start=(k == 0),
                        stop=(k == KC - 1),
                    )
                eps3 = eps.rearrange("p (b w) -> p b w", w=W)
                hterm = hcw1t_sb[:, h, c * 8:(c + 1) * 8].unsqueeze(2)
                nc.vector.tensor_tensor(
                    out=eps3, in0=eps3,
                    in1=hterm.to_broadcast([128, 8, W]),
                    op=mybir.AluOpType.add,
                )
                eq = eqp.tile([128, 512], MMDT, tag="eq_sb", name=f"eq{h}_{c}")
                nc.scalar.activation(
                    out=eq,
                    in_=eps,
                    func=mybir.ActivationFunctionType.Tanh,
                    bias=bias_sb[:, h:h + 1],
                    scale=1.0,
                )
                nc.tensor.matmul(
                    out=scores_ps[:, csl],
                    lhsT=v_sb[:, h:h + 1],
                    rhs=eq,
                    start=(h == 0),
                    stop=(h == HOC - 1),
                )

        # ---- softmax over each sample's 64 window positions -----------------
        scores_fl = consts.tile([1, BW], F32, name="scores_fl")
        nc.scalar.copy(out=scores_fl, in_=scores_ps)
        sc32 = consts.tile([BC, W], F32, name="sc32")
        nc.sync.dma_start(out=sc32, in_=scores_fl)  # [1,2048] -> [32,64]
        negmx = consts.tile([BC, 1], F32, name="negmx")
        nc.vector.tensor_reduce(
            out=negmx, in_=sc32, axis=mybir.AxisListType.X,
            op=mybir.AluOpType.max, negate=True,
        )
        attn32 = consts.tile([BC, W], F32, name="attn32")
        sume = consts.tile([BC, 1], F32, name="sume")
        nc.scalar.activation(
            out=attn32, in_=sc32, func=mybir.ActivationFunctionType.Exp,
            bias=negmx, scale=1.0, accum_out=sume,
        )
        rs = consts.tile([BC, 1], F32, name="rs")
        nc.vector.reciprocal(out=rs, in_=sume)
        nc.vector.tensor_scalar_mul(attn32, attn32, rs)
        nc.sync.dma_start(out=io["attn_d"], in_=attn32)

        # replicate attn over all 128 partitions (K=1 matmul against ones)
        attn_fl = consts.tile([1, BW], F32, name="attn_fl")
        nc.sync.dma_start(out=attn_fl, in_=attn32)  # [32,64] -> [1,2048]
        arep_sb = consts.tile([128, BW], F32, name="arep_sb")
        for c in range(NQ):
            csl = slice(c * 512, (c + 1) * 512)
            aps = psq.tile([128, 512], F32, tag="eq", name=f"aps{c}")
            nc.tensor.matmul(
                out=aps, lhsT=ones_sb, rhs=attn_fl[:, csl],
                start=True, stop=True,
            )
            nc.scalar.copy(out=arep_sb[:, csl], in_=aps)

        # ---- context: segmented weighted sum on the DVE ---------------------
        # ctxT[ho, b] = sum_w MtT[ho, (b,w)] * attn[(b,w)]
        ctxT_sb = consts.tile([128, KC, BC], F32, name="ctxT_sb")
        for k in range(KC):
            for c in range(NQ):
                csl = slice(c * 512, (c + 1) * 512)
                tmp = ctp.tile([128, 512], F32, tag="ctmp", name=f"ctmp{k}_{c}")
                nc.vector.tensor_tensor(
                    out=tmp, in0=mt[k, c].bitcast(F32), in1=arep_sb[:, csl],
                    op=mybir.AluOpType.mult,
                )
                nc.vector.tensor_reduce(
                    out=ctxT_sb[:, k, c * 8:(c + 1) * 8],
                    in_=tmp.rearrange("p (b w) -> p b w", w=W),
                    axis=mybir.AxisListType.X,
                    op=mybir.AluOpType.add,
                )

        # transpose [ho, b] -> [b, ho] via PE, then store
        ctx_sb = consts.tile([BC, H], F32, name="ctx_sb")
        for k in range(KC):
            tps = psq.tile([BC, 128], F32, tag="eq", name=f"tps{k}")
            nc.tensor.transpose(out=tps, in_=ctxT_sb[:, k, :], identity=ident_sb)
            nc.scalar.copy(out=ctx_sb[:, k * 128:(k + 1) * 128], in_=tps)
        nc.sync.dma_start(out=io["ctx_d"], in_=ctx_sb)


_CACHE = {}


def _get_program():
    if "nc" not in _CACHE:
        _CACHE["nc"] = _build_program()
    return _CACHE["nc"]


def _host_prep(inputs, hc, W_attn, b_attn, v, setence_len, parent):
    """Index-based gathers + layout transposes (no model FLOPs)."""
    sl = np.asarray(setence_len).astype(np.int64)
    pr = np.asarray(parent).astype(np.int64)
    x = np.ascontiguousarray(np.asarray(inputs, dtype=np.float32))
    hc = np.asarray(hc, dtype=np.float32)
    W_attn = np.asarray(W_attn, dtype=np.float32)
    b_attn = np.asarray(b_attn, dtype=np.float32)
    v = np.asarray(v, dtype=np.float32)

    starts = sl - W
    rows = starts[:, None] + np.arange(W)[None, :]            # [B, W]
    win = x[np.arange(B)[:, None], rows, :]                   # [B, W, H] f32

    # shared (replicated) tensors
    WT = np.ascontiguousarray(W_attn.T)                       # [2H, H]
    w1c = np.ascontiguousarray(WT[:H].reshape(KC, 128, H))
    w2c = np.ascontiguousarray(WT[H:].reshape(KC, 128, H))
    biasc = np.ascontiguousarray(b_attn.reshape(HOC, 128).T)  # [128, 8]
    vc = np.ascontiguousarray(v.reshape(HOC, 128).T)          # [128, 8]

    in_maps = []
    for m in range(NCORES):
        s = slice(m * BC, (m + 1) * BC)
        wcore = win[s].reshape(BW, H)                         # [2048, 1024]
        wT = wcore.T                                          # [1024, 2048] view
        wt = np.ascontiguousarray(
            wT.reshape(KC, 128, NQ, 512).transpose(0, 2, 1, 3)
        )                                                     # [8, 4, 128, 512]
        hct = np.ascontiguousarray(
            hc[s].T.reshape(KC, 128, BC).transpose(1, 0, 2)
        )                                                     # [128, 8, 32]
        pidx = (np.arange(BC, dtype=np.int64) * W + (W - 1 - pr[s])).astype(
            np.int32
        ).reshape(BC, 1)
        in_maps.append(
            {
                "wt": wt,
                "wnat": np.ascontiguousarray(wcore),
                "hct": hct,
                "pidx": pidx,
                "w1c": w1c,
                "w2c": w2c,
                "biasc": biasc,
                "vc": vc,
            }
        )
    return in_maps


def kernel(inputs, hc, W_attn, b_attn, v, setence_len, parent, _trace=False):
    assert inputs.shape == (B, S, H)
    in_maps = _host_prep(inputs, hc, W_attn, b_attn, v, setence_len, parent)
    nc = _get_program()
    res = run_bass_kernel_spmd(nc, in_maps, core_ids=list(range(NCORES)),
                               trace=_trace)
    attn = np.concatenate([r["attn_o"] for r in res.results], axis=0)
    ctxo = np.concatenate([r["ctx_o"] for r in res.results], axis=0)
    paro = np.concatenate([r["par_o"] for r in res.results], axis=0)
    out = (
        attn.reshape(B, 1, W).astype(np.float32),
        ctxo.reshape(B, 1, H).astype(np.float32),
        paro.reshape(B, H).astype(np.float32),
    )
    if _trace:
        _CACHE["last_results"] = res
    return out
